# revision 11
# baseline (speedup 1.0000x reference)
"""Trainium2 Bass kernel for CompactS6Layer (Mamba/S6-style selective-scan block).

Sharding: data-parallel over batch B=8 -> one batch element per NeuronCore
(8 cores), weights replicated, no collectives.

Two-phase streaming kernel per core:
  Phase 1 (projections): LayerNorm -> PE-transpose to feature-major ->
    x_proj = xn @ W_in_gate (x_in / silu(z)), dt = softplus(x_in @ W_dt + b),
    B_t / C_t projections, dt*x_in; feature-major intermediates spilled to DRAM.
  Phase 2 (scan + output): per 256-step chunk, per 128-feature group:
    dA = exp(dt * A[d,n]) (ACT with per-partition scale), u = (dt*x_in)*B_t
    (broadcast APs), h = tensor_tensor_scan(dA, u) (native DVE first-order
    recurrence, chained across chunks via `initial`), y = sum_n h*C_t,
    ssm = y*silu(z) + x_in*D, out = ssm @ W_out, transpose back, +x, store.
"""

import numpy as np
from contextlib import ExitStack

D = 1024
DSTATE = 8
BATCH = 8
L = 2048
EPS = 1e-5
GD = D // 128          # 8 feature groups of 128
TC = 256               # time chunk
NCH = L // TC          # 8 chunks
NT = TC // 128         # t-tiles per chunk
NO2 = (2 * D) // 128   # 16 output blocks for in_gate proj
FP32R = True           # fast fp32 matmul mode

_CACHE = {}


def _build():
    import concourse.tile as tile
    import concourse.mybir as mybir
    from concourse import bacc

    f32 = mybir.dt.float32
    f32r = mybir.dt.float32r
    i32 = mybir.dt.int32
    AF = mybir.ActivationFunctionType
    ALU = mybir.AluOpType
    AX = mybir.AxisListType

    fr = f32r if FP32R else f32

    def mmcast(ap):
        return ap

    nc = bacc.Bacc("TRN2", target_bir_lowering=False, debug=False)

    x_d = nc.dram_tensor("x", [L, D], f32, kind="ExternalInput").ap()
    h0_d = nc.dram_tensor("h0", [D, DSTATE], f32, kind="ExternalInput").ap()
    win_d = nc.dram_tensor("w_in", [D, 2 * D], fr, kind="ExternalInput").ap()
    wdt_d = nc.dram_tensor("w_dt", [D, D], fr, kind="ExternalInput").ap()
    bdt_d = nc.dram_tensor("b_dt", [D], f32, kind="ExternalInput").ap()
    a_d = nc.dram_tensor("a_mat", [D, DSTATE], f32, kind="ExternalInput").ap()
    wb_d = nc.dram_tensor("w_b", [D, DSTATE], fr, kind="ExternalInput").ap()
    wc_d = nc.dram_tensor("w_c", [D, DSTATE], fr, kind="ExternalInput").ap()
    dp_d = nc.dram_tensor("d_param", [D], f32, kind="ExternalInput").ap()
    wout_d = nc.dram_tensor("w_out", [D, D], fr, kind="ExternalInput").ap()
    xo_d = nc.dram_tensor("x_out", [L, D], f32, kind="ExternalOutput").ap()
    ho_d = nc.dram_tensor("h_out", [D, DSTATE], f32, kind="ExternalOutput").ap()
    # feature-major [D, L] spill buffers between the two phases
    xin_s = nc.dram_tensor("xin_spill", [D, L], f32).ap()
    slz_s = nc.dram_tensor("slz_spill", [D, L], f32).ap()
    dts_s = nc.dram_tensor("dt_spill", [D, L], f32).ap()
    dxi_s = nc.dram_tensor("dxi_spill", [D, L], f32).ap()
    zraw_s = nc.dram_tensor("zraw_spill", [D, L], f32).ap()
    b_dram = nc.dram_tensor("b_dram", [DSTATE, L], f32).ap()
    c_dram = nc.dram_tensor("c_dram", [DSTATE, L], f32).ap()

    with tile.TileContext(nc) as tc, ExitStack() as ctx:
        # ---- outer pools (live across both phases)
        p_const = ctx.enter_context(tc.tile_pool(name="const", bufs=1))
        p_stat = ctx.enter_context(tc.tile_pool(name="stat", bufs=12))
        p_mm = ctx.enter_context(tc.tile_pool(name="psmm", bufs=3, space="PSUM"))
        p_tp = ctx.enter_context(tc.tile_pool(name="pstp", bufs=3, space="PSUM"))

        eps_sb = p_const.tile([128, 1], f32)
        nc.vector.memset(eps_sb[:], EPS)
        one_sb = p_const.tile([128, 1], f32)
        nc.vector.memset(one_sb[:], 1.0)
        ident_i = p_const.tile([128, 128], i32)
        nc.gpsimd.iota(ident_i[:], pattern=[[1, 128]], base=0, channel_multiplier=-1)
        ident = p_const.tile([128, 128], f32)
        nc.vector.tensor_scalar(
            out=ident[:], in0=ident_i[:], scalar1=0, scalar2=None, op0=ALU.is_equal
        )
        bdt_sb = p_const.tile([128, GD], f32)
        nc.sync.dma_start(bdt_sb[:], bdt_d.rearrange("(g p) -> p g", p=128))
        dp_sb = p_const.tile([128, GD], f32)
        nc.sync.dma_start(dp_sb[:], dp_d.rearrange("(g p) -> p g", p=128))
        a_sb = p_const.tile([128, GD, DSTATE], f32)
        nc.sync.dma_start(a_sb[:], a_d.rearrange("(g p) n -> p g n", p=128))
        h0_sb = p_const.tile([128, GD, DSTATE], f32)
        nc.sync.dma_start(h0_sb[:], h0_d.rearrange("(g p) n -> p g n", p=128))

        # ================= PHASE 1a: LN + x_proj =================
        with ExitStack() as c1:
            p_w1 = c1.enter_context(tc.tile_pool(name="w1", bufs=1))
            p_scr = c1.enter_context(tc.tile_pool(name="scratch", bufs=2))
            p_x = c1.enter_context(tc.tile_pool(name="xa", bufs=3))
            p_xnt = c1.enter_context(tc.tile_pool(name="xnt", bufs=2 * GD))
            p_fm = c1.enter_context(tc.tile_pool(name="fm", bufs=9))
            p_fm2 = c1.enter_context(tc.tile_pool(name="fm2", bufs=9))

            w_in = [p_w1.tile([128, 2 * D], fr, tag=f"w_in{g}", name=f"w_in{g}") for g in range(GD)]
            for g in range(GD):
                nc.sync.dma_start(w_in[g][:], win_d[g * 128 : (g + 1) * 128, :])

            for ch in range(NCH):
                t0 = ch * TC
                xn_tiles = []
                for tt in range(NT):
                    xt = p_x.tile([128, D], f32, tag="x1")
                    nc.sync.dma_start(
                        xt[:], x_d[t0 + tt * 128 : t0 + (tt + 1) * 128, :]
                    )
                    ssum = p_stat.tile([128, 1], f32, tag="ssum")
                    nc.vector.tensor_reduce(ssum[:], xt[:], axis=AX.X, op=ALU.add)
                    scr = p_scr.tile([128, D], f32)
                    sumsq = p_stat.tile([128, 1], f32, tag="sumsq")
                    nc.scalar.activation(scr[:], xt[:], AF.Square, accum_out=sumsq[:])
                    mu = p_stat.tile([128, 1], f32, tag="mu")
                    nc.scalar.mul(mu[:], ssum[:], 1.0 / D)
                    ex2 = p_stat.tile([128, 1], f32, tag="ex2")
                    nc.scalar.mul(ex2[:], sumsq[:], 1.0 / D)
                    mu2 = p_stat.tile([128, 1], f32, tag="mu2")
                    nc.vector.tensor_tensor(mu2[:], mu[:], mu[:], op=ALU.mult)
                    var = p_stat.tile([128, 1], f32, tag="var")
                    nc.vector.tensor_tensor(var[:], ex2[:], mu2[:], op=ALU.subtract)
                    lnv = p_stat.tile([128, 1], f32, tag="lnv")
                    nc.scalar.activation(lnv[:], var[:], AF.Ln, bias=eps_sb[:])
                    rstd = p_stat.tile([128, 1], f32, tag="rstd")
                    nc.scalar.activation(rstd[:], lnv[:], AF.Exp, scale=-0.5)
                    xn = p_x.tile([128, D], f32, tag="xn")
                    nc.vector.tensor_scalar(
                        out=xn[:], in0=xt[:], scalar1=mu[:], scalar2=rstd[:],
                        op0=ALU.subtract, op1=ALU.mult,
                    )
                    xn_tiles.append(xn)

                xnt = [p_xnt.tile([128, TC], fr, name="xnt") for _ in range(GD)]
                for g in range(GD):
                    for tt in range(NT):
                        pt = p_tp.tile([128, 128], f32, tag="pt")
                        nc.tensor.transpose(
                            pt[:], xn_tiles[tt][:, g * 128 : (g + 1) * 128], ident[:]
                        )
                        nc.scalar.copy(xnt[g][:, tt * 128 : (tt + 1) * 128], pt[:])

                for o in range(NO2):
                    ps = p_mm.tile([128, TC], f32, tag="ps")
                    for g in range(GD):
                        nc.tensor.matmul(
                            ps[:],
                            mmcast(w_in[g][:, o * 128 : (o + 1) * 128]),
                            mmcast(xnt[g][:]),
                            start=(g == 0),
                            stop=(g == GD - 1),
                        )
                    if o < GD:
                        xin = p_fm.tile([128, TC], f32)
                        nc.scalar.copy(xin[:], ps[:])
                        nc.sync.dma_start(
                            xin_s[o * 128 : (o + 1) * 128, t0 : t0 + TC], xin[:]
                        )
                    else:
                        og = o - GD
                        zr = p_fm2.tile([128, TC], f32)
                        nc.scalar.copy(zr[:], ps[:])
                        nc.sync.dma_start(
                            zraw_s[og * 128 : (og + 1) * 128, t0 : t0 + TC], zr[:]
                        )

        # ================= PHASE 1b: dt / B / C =================
        with ExitStack() as c1:
            p_w1b = c1.enter_context(tc.tile_pool(name="w1b", bufs=1))
            p_xi = c1.enter_context(tc.tile_pool(name="xi", bufs=2 * GD))
            p_fm3 = c1.enter_context(tc.tile_pool(name="fm3", bufs=9))
            p_fm4 = c1.enter_context(tc.tile_pool(name="fm4", bufs=9))
            p_bcs = c1.enter_context(tc.tile_pool(name="bcs", bufs=2))
            p_z = c1.enter_context(tc.tile_pool(name="z", bufs=3))
            p_ez = c1.enter_context(tc.tile_pool(name="ez", bufs=2))
            p_bcp = c1.enter_context(tc.tile_pool(name="psbc", bufs=2, space="PSUM"))

            w_dt = [p_w1b.tile([128, D], fr, tag=f"w_dt{g}", name=f"w_dt{g}") for g in range(GD)]
            w_b = [p_w1b.tile([128, DSTATE], fr, tag=f"w_b{g}", name=f"w_b{g}") for g in range(GD)]
            w_c = [p_w1b.tile([128, DSTATE], fr, tag=f"w_c{g}", name=f"w_c{g}") for g in range(GD)]
            for g in range(GD):
                sl = slice(g * 128, (g + 1) * 128)
                nc.sync.dma_start(w_dt[g][:], wdt_d[sl, :])
                nc.sync.dma_start(w_b[g][:], wb_d[sl, :])
                nc.sync.dma_start(w_c[g][:], wc_d[sl, :])

            for ch in range(NCH):
                t0 = ch * TC
                xin_t = [p_xi.tile([128, TC], fr, name="xin_t") for _ in range(GD)]
                for g in range(GD):
                    nc.sync.dma_start(
                        xin_t[g][:],
                        xin_s[g * 128 : (g + 1) * 128, t0 : t0 + TC].bitcast(fr),
                    )

                for o in range(GD):
                    ps = p_mm.tile([128, TC], f32, tag="ps")
                    for g in range(GD):
                        nc.tensor.matmul(
                            ps[:],
                            mmcast(w_dt[g][:, o * 128 : (o + 1) * 128]),
                            mmcast(xin_t[g][:]),
                            start=(g == 0),
                            stop=(g == GD - 1),
                        )
                    edt = p_fm3.tile([128, TC], f32, tag="edt")
                    nc.scalar.activation(
                        edt[:], ps[:], AF.Exp, bias=bdt_sb[:, o : o + 1]
                    )
                    dtt = p_fm3.tile([128, TC], f32)
                    nc.scalar.activation(dtt[:], edt[:], AF.Ln, bias=one_sb[:])
                    nc.sync.dma_start(
                        dts_s[o * 128 : (o + 1) * 128, t0 : t0 + TC], dtt[:]
                    )
                    dxi = p_fm4.tile([128, TC], f32)
                    nc.vector.tensor_tensor(dxi[:], dtt[:], xin_t[o][:].bitcast(f32), op=ALU.mult)
                    nc.sync.dma_start(
                        dxi_s[o * 128 : (o + 1) * 128, t0 : t0 + TC], dxi[:]
                    )
                    zr = p_z.tile([128, TC], f32, name="zr")
                    nc.sync.dma_start(
                        zr[:], zraw_s[o * 128 : (o + 1) * 128, t0 : t0 + TC]
                    )
                    ez = p_ez.tile([128, TC], f32, tag="ez", name="ez")
                    nc.scalar.activation(ez[:], zr[:], AF.Exp, scale=-1.0)
                    dsg = p_ez.tile([128, TC], f32, tag="dsg", name="dsg")
                    nc.vector.tensor_scalar(
                        out=dsg[:], in0=ez[:], scalar1=1.0, scalar2=None, op0=ALU.add
                    )
                    rsg = p_ez.tile([128, TC], f32, tag="rsg", name="rsg")
                    nc.vector.reciprocal(rsg[:], dsg[:])
                    slz = p_ez.tile([128, TC], f32, tag="slz", name="slz")
                    nc.vector.tensor_tensor(slz[:], zr[:], rsg[:], op=ALU.mult)
                    nc.sync.dma_start(
                        slz_s[o * 128 : (o + 1) * 128, t0 : t0 + TC], slz[:]
                    )

                for w_sb, dst in ((w_b, b_dram), (w_c, c_dram)):
                    ps = p_bcp.tile([DSTATE, TC], f32, tag="psbc")
                    for g in range(GD):
                        nc.tensor.matmul(
                            ps[:],
                            mmcast(w_sb[g][:]),
                            mmcast(xin_t[g][:]),
                            start=(g == 0),
                            stop=(g == GD - 1),
                        )
                    stg = p_bcs.tile([DSTATE, TC], f32, tag="stg", name="stg")
                    nc.scalar.copy(stg[:], ps[:])
                    nc.sync.dma_start(dst[:, t0 : t0 + TC], stg[:])

        # ================= PHASE 2 =================
        with ExitStack() as c2:
            p_w2 = c2.enter_context(tc.tile_pool(name="w2", bufs=1))
            p_x = c2.enter_context(tc.tile_pool(name="x2", bufs=4))
            p_in2 = c2.enter_context(tc.tile_pool(name="in2", bufs=8))
            p_rep = c2.enter_context(tc.tile_pool(name="rep", bufs=1))
            p_dA = c2.enter_context(tc.tile_pool(name="dA", bufs=2))
            p_u = c2.enter_context(tc.tile_pool(name="u", bufs=2))
            p_h = c2.enter_context(tc.tile_pool(name="h", bufs=2))
            p_carry = c2.enter_context(tc.tile_pool(name="carry", bufs=2 * GD))
            p_y = c2.enter_context(tc.tile_pool(name="y", bufs=2))
            p_ssm = c2.enter_context(tc.tile_pool(name="ssm", bufs=GD + 1))
            p_out = c2.enter_context(tc.tile_pool(name="outfm", bufs=4))
            p_xo = c2.enter_context(tc.tile_pool(name="xo", bufs=3))

            w_out = [p_w2.tile([128, D], fr, tag=f"w_out{g}", name=f"w_out{g}") for g in range(GD)]
            for g in range(GD):
                nc.sync.dma_start(w_out[g][:], wout_d[g * 128 : (g + 1) * 128, :])

            carry = [None] * GD
            for ch in range(NCH):
                t0 = ch * TC
                dt2 = [p_in2.tile([128, TC], f32, tag="dt2", name="dt2") for _ in range(GD)]
                dxi2 = [p_in2.tile([128, TC], f32, tag="dxi2", name="dxi2") for _ in range(GD)]
                xin2 = [p_in2.tile([128, TC], f32, tag="xin2", name="xin2") for _ in range(GD)]
                slz2 = [p_in2.tile([128, TC], f32, tag="slz2", name="slz2") for _ in range(GD)]
                for g in range(GD):
                    sl = slice(g * 128, (g + 1) * 128)
                    nc.sync.dma_start(dt2[g][:], dts_s[sl, t0 : t0 + TC])
                    nc.sync.dma_start(dxi2[g][:], dxi_s[sl, t0 : t0 + TC])
                    nc.sync.dma_start(xin2[g][:], xin_s[sl, t0 : t0 + TC])
                    nc.sync.dma_start(slz2[g][:], slz_s[sl, t0 : t0 + TC])
                x2 = []
                for tt in range(NT):
                    xt = p_x.tile([128, D], f32, tag="x2")
                    nc.sync.dma_start(
                        xt[:], x_d[t0 + tt * 128 : t0 + (tt + 1) * 128, :]
                    )
                    x2.append(xt)

                b_rep = p_rep.tile([128, DSTATE, TC], f32, tag="b_rep")
                c_rep = p_rep.tile([128, DSTATE, TC], f32, tag="c_rep")
                for n in range(DSTATE):
                    nc.sync.dma_start(
                        b_rep[:, n, :],
                        b_dram[n : n + 1, t0 : t0 + TC].partition_broadcast(128),
                    )
                    nc.sync.dma_start(
                        c_rep[:, n, :],
                        c_dram[n : n + 1, t0 : t0 + TC].partition_broadcast(128),
                    )

                ssm_t = []
                for g in range(GD):
                    dA = p_dA.tile([128, DSTATE, TC], f32)
                    for n in range(DSTATE):
                        nc.scalar.activation(
                            dA[:, n, :], dt2[g][:], AF.Exp,
                            scale=a_sb[:, g, n : n + 1],
                        )
                    u = p_u.tile([128, DSTATE, TC], f32, tag="u")
                    nc.vector.tensor_tensor(
                        u[:],
                        dxi2[g][:][:, None, :].broadcast_to([128, DSTATE, TC]),
                        b_rep[:],
                        op=ALU.mult,
                    )
                    h = p_h.tile([128, DSTATE, TC], f32)
                    for n in range(DSTATE):
                        init = (
                            h0_sb[:, g, n : n + 1]
                            if ch == 0
                            else carry[g][:, n, :]
                        )
                        nc.vector.tensor_tensor_scan(
                            h[:, n, :], dA[:, n, :], u[:, n, :], init,
                            op0=ALU.mult, op1=ALU.add,
                        )
                    cnew = p_carry.tile([128, DSTATE, 1], f32)
                    nc.scalar.copy(cnew[:], h[:, :, TC - 1 : TC])
                    carry[g] = cnew
                    hc = p_u.tile([128, DSTATE, TC], f32, tag="u")
                    nc.gpsimd.tensor_tensor(hc[:], h[:], c_rep[:], op=ALU.mult)
                    y = p_y.tile([128, TC], f32, tag="y")
                    nc.vector.tensor_reduce(
                        y[:], hc[:].rearrange("p n t -> p t n"), axis=AX.X,
                        op=ALU.add,
                    )
                    t1 = p_y.tile([128, TC], f32, tag="t1")
                    nc.vector.tensor_tensor(t1[:], y[:], slz2[g][:], op=ALU.mult)
                    xd = p_y.tile([128, TC], f32, tag="xd")
                    nc.gpsimd.tensor_scalar_mul(xd[:], xin2[g][:], dp_sb[:, g : g + 1])
                    ssm = p_y.tile([128, TC], f32, tag="ssmf")
                    nc.vector.tensor_tensor(ssm[:], t1[:], xd[:], op=ALU.add)
                    ssm_r = p_ssm.tile([128, TC], fr, tag="ssm")
                    nc.scalar.copy(ssm_r[:], ssm[:])
                    ssm_t.append(ssm_r)

                xo_t = [p_xo.tile([128, D], f32, name="xo_t") for _ in range(NT)]
                for o in range(GD):
                    ps = p_mm.tile([128, TC], f32, tag="ps")
                    for g in range(GD):
                        nc.tensor.matmul(
                            ps[:],
                            mmcast(w_out[g][:, o * 128 : (o + 1) * 128]),
                            mmcast(ssm_t[g][:]),
                            start=(g == 0),
                            stop=(g == GD - 1),
                        )
                    ofm = p_out.tile([128, TC], f32)
                    nc.scalar.copy(ofm[:], ps[:])
                    for tt in range(NT):
                        pt = p_tp.tile([128, 128], f32, tag="pt")
                        nc.tensor.transpose(
                            pt[:], ofm[:, tt * 128 : (tt + 1) * 128], ident[:]
                        )
                        nc.vector.tensor_tensor(
                            xo_t[tt][:, o * 128 : (o + 1) * 128],
                            pt[:],
                            x2[tt][:, o * 128 : (o + 1) * 128],
                            op=ALU.add,
                        )
                for tt in range(NT):
                    nc.sync.dma_start(
                        xo_d[t0 + tt * 128 : t0 + (tt + 1) * 128, :], xo_t[tt][:]
                    )

            hf = p_const.tile([128, GD, DSTATE], f32)
            for g in range(GD):
                nc.scalar.copy(hf[:, g, :], carry[g][:, :, 0])
            nc.sync.dma_start(ho_d.rearrange("(g p) n -> p g n", p=128), hf[:])

    nc.compile()
    return nc


def _get_nc():
    if "nc" not in _CACHE:
        _CACHE["nc"] = _build()
    return _CACHE["nc"]


def kernel(x, h_prev, gamma, beta, W_in_gate, W_dt, b_dt, A_log, W_B, W_C,
           D_param, W_out):
    from concourse.bass_utils import run_bass_kernel_spmd

    x = np.asarray(x, np.float32)
    h_prev = np.asarray(h_prev, np.float32)
    gamma = np.asarray(gamma, np.float32)
    beta = np.asarray(beta, np.float32)
    # gamma folds exactly into W_in_gate: (xn*gamma) @ W == xn @ (gamma[:,None]*W)
    w_in_eff = np.ascontiguousarray(np.asarray(W_in_gate, np.float32)
                                    * gamma[:, None])
    assert np.all(beta == 0.0), "beta!=0 not supported by this kernel build"
    a_mat = -np.exp(np.asarray(A_log, np.float32))
    common = {
        "w_in": w_in_eff,
        "w_dt": np.ascontiguousarray(np.asarray(W_dt, np.float32)),
        "b_dt": np.ascontiguousarray(np.asarray(b_dt, np.float32)),
        "a_mat": np.ascontiguousarray(a_mat),
        "w_b": np.ascontiguousarray(np.asarray(W_B, np.float32)),
        "w_c": np.ascontiguousarray(np.asarray(W_C, np.float32)),
        "d_param": np.ascontiguousarray(np.asarray(D_param, np.float32)),
        "w_out": np.ascontiguousarray(np.asarray(W_out, np.float32)),
    }
    in_maps = []
    for b in range(BATCH):
        m = dict(common)
        m["x"] = np.ascontiguousarray(x[b])
        m["h0"] = np.ascontiguousarray(h_prev[b].reshape(D, DSTATE))
        in_maps.append(m)

    nc = _get_nc()
    res = run_bass_kernel_spmd(nc, in_maps, list(range(BATCH))).results
    x_out = np.stack([res[b]["x_out"] for b in range(BATCH)])
    h_final = np.stack([res[b]["h_out"].reshape(D * DSTATE) for b in range(BATCH)])
    return x_out, h_final
